# revision 13
# baseline (speedup 1.0000x reference)
"""CrossCosineEmbeddingLoss kernel for 8 trn2 NeuronCores (v7).

loss = mean over all (i,j) of: 1 - cos(x_i, y_j) if i==j else relu(cos(x_i, y_j))

Identity:  total = sum_ij relu(sim_ij) + sum_i (1 - sim_ii - relu(sim_ii))

Sharding (2x4 grid): core c = (bi, bj), bi = c // 2, bj = c % 2.
  x rows [2048*bi, 2048*(bi+1)) x y rows [4096*bj, 4096*(bj+1)).
Each core computes sum_ij relu(x_hat_i . y_j) / ||y_j|| over its block.
Diag correction only used from cores whose x block lies in their y range.

Engine queues are strictly in-order, so emission order is chosen so no
early-queue op waits on a long dependency chain (e.g. rny sqrt ops are
emitted right before their first consumer tile).

Per-core pipeline:
  - x: SWDGE cast-DMA to bf16 (2 halves); DVE STT sumsq; rsqrt; DVE
    scale; PE transpose; ACT copy -> x_hatT   (pipelined per half)
  - y: SWDGE cast-DMA to bf16 (2 chunks); GpSimd squares (natural) + DVE
    segmented reduce + rsqrt -> rny (per chunk); PE transpose + DVE copy
    -> yT
  - main: 32 j-tiles of [128j, 2048i] fp32 PSUM; two 2-bank pool tiles
    per j-tile (bufs=2 each): ACT activation(Relu, scale=rny, accum_out)
    on one, DVE tensor_scalar(max 0, accum_out) on the other, outputs
    dumped to SBUF bf16 scratch (in-place PSUM write is slower).
  - diag (after main pools close): prodT = x_hatT * ydT, ydT^2 (GpSimd);
    per-column sums via N=1 ones-matmuls into one PSUM bank; small ops.
Host combines [128,2] partials; diag col used only from owner cores.
"""

import numpy as np

import concourse.bacc as bacc
import concourse.bass as bass
import concourse.tile as tile
from concourse import mybir
from concourse.bass_utils import run_bass_kernel_spmd
from concourse.masks import make_identity

N, D = 8192, 128
NCORES = 8
XI = 2048            # x rows per core
YJ = 4096            # y rows per core
TXI = XI // 128      # 16 x tiles
TYJ = YJ // 128      # 32 y j-tiles
YCH = 2              # y cast-DMA chunks
YCT = TYJ // YCH     # 16 j-tiles per chunk
ACW = 1024           # ACT's share of each [128, 2048] tile (bank aligned)
RNY1_AT = 8          # emit chunk-1 norm tail before this main tile
MID_AT = 6           # emit diag prodT before this main tile

f32 = mybir.dt.float32
bf16 = mybir.dt.bfloat16
AF = mybir.ActivationFunctionType
ALU = mybir.AluOpType
AX = mybir.AxisListType

_CACHE = {}


def _build():
    if "nc" in _CACHE:
        return _CACHE["nc"]
    nc = bacc.Bacc("TRN2", target_bir_lowering=False, debug=False,
                   num_devices=NCORES)
    xs_d = nc.dram_tensor("xs", [XI, D], f32, kind="ExternalInput")
    y_d = nc.dram_tensor("y", [YJ, D], f32, kind="ExternalInput")
    yd_d = nc.dram_tensor("yd", [XI, D], f32, kind="ExternalInput")
    out_d = nc.dram_tensor("out", [128, 2], f32, kind="ExternalOutput")

    with tile.TileContext(nc) as tc:
        with (
            tc.tile_pool(name="singles", bufs=1) as singles,
            tc.tile_pool(name="scr", bufs=2) as scr,
        ):
            ident = singles.tile([128, 128], bf16)
            make_identity(nc, ident[:])
            onecol = singles.tile([128, 1], bf16)
            nc.vector.memset(onecol[:], 1.0)
            warm = singles.tile([128, 1], f32)
            nc.vector.memset(warm[:], 1.0)
            nc.scalar.sqrt(warm[:], warm[:])   # preload sqrt table set early

            ynat = singles.tile([128, TYJ, 128], bf16)   # row j: 2048g+16p+t
            yT = singles.tile([128, TYJ, 128], bf16)     # [d, t, j-col]
            xnat = singles.tile([128, TXI, 128], bf16)   # row i: 1024h+8p+t
            xhat = singles.tile([128, TXI, 128], bf16)
            xhatT = singles.tile([128, TXI * 128], bf16)
            ydn = singles.tile([128, TXI, 128], bf16)
            ydT = singles.tile([128, TXI * 128], bf16)
            prodT = singles.tile([128, TXI * 128], bf16)

            ny2 = singles.tile([128, TYJ], f32)
            t2y = singles.tile([128, TYJ], f32)
            rny = singles.tile([128, TYJ], f32)
            nx2 = singles.tile([128, TXI], f32)
            t1x = singles.tile([128, TXI], f32)
            rnx = singles.tile([128, TXI], f32)
            nd64 = singles.tile([128, 64], f32)
            rnyd = singles.tile([128, TXI], f32)
            sim_d = singles.tile([128, TXI], f32)
            relu_d = singles.tile([128, TXI], f32)
            R = singles.tile([128, 64], f32)
            outsb = singles.tile([128, 2], f32)
            dumpA = singles.tile([128, ACW], bf16)
            dumpD = singles.tile([128, 2048 - ACW], bf16)

            # ---- input DMAs (SWDGE cast fp32->bf16): x, y0, yd, y1
            for h in range(2):
                rows = slice(1024 * h, 1024 * (h + 1))
                nc.gpsimd.dma_start(
                    out=xnat[:, 8 * h:8 * (h + 1), :],
                    in_=xs_d[rows].rearrange("(p t) d -> p t d", t=8))
            nc.gpsimd.dma_start(
                out=ynat[:, 0:YCT, :],
                in_=y_d[0:2048].rearrange("(p t) d -> p t d", t=YCT))
            for h in range(2):
                rows = slice(1024 * h, 1024 * (h + 1))
                nc.gpsimd.dma_start(
                    out=ydn[:, 8 * h:8 * (h + 1), :],
                    in_=yd_d[rows].rearrange("(p t) d -> p t d", t=8))
            nc.gpsimd.dma_start(
                out=ynat[:, YCT:TYJ, :],
                in_=y_d[2048:4096].rearrange("(p t) d -> p t d", t=YCT))

            # ---- x norms + scale (per half); DVE STT sumsq on bf16
            for h in range(2):
                hs = slice(8 * h, 8 * (h + 1))
                for t in range(8 * h, 8 * h + 8):
                    nc.vector.scalar_tensor_tensor(
                        out=scr.tile([128, 128], bf16, tag="st",
                                     name="st")[:],
                        in0=xnat[:, t, :], scalar=1.0, in1=xnat[:, t, :],
                        op0=ALU.mult, op1=ALU.mult,
                        accum_out=nx2[:, t:t + 1])
                nc.vector.reciprocal(t1x[:, hs], nx2[:, hs])
                nc.scalar.sqrt(rnx[:, hs], t1x[:, hs])   # 1/||x_i||
                for t in range(8 * h, 8 * h + 8):
                    nc.vector.tensor_scalar(
                        out=xhat[:, t, :], in0=xnat[:, t, :],
                        scalar1=rnx[:, t:t + 1], scalar2=None,
                        op0=ALU.mult)

            # ---- GpSimd natural squares for y norms (both chunks)
            ysq = []
            with nc.allow_low_precision("norm sums in bf16 are plenty"):
                for g in range(YCH):
                    s = scr.tile([128, YCT, 128], bf16, tag=f"sq{g}",
                                 name=f"ysq{g}")
                    gs = slice(YCT * g, YCT * (g + 1))
                    nc.gpsimd.tensor_mul(
                        s[:].rearrange("p a b -> p (a b)"),
                        ynat[:, gs, :].rearrange("p a b -> p (a b)"),
                        ynat[:, gs, :].rearrange("p a b -> p (a b)"))
                    ysq.append(s)

            # ---- transposes on PE (bf16) + copies to SBUF
            with tc.tile_pool(name="tpsum", bufs=2, space="PSUM") as tpsum:
                for h in range(2):
                    ptx = tpsum.tile([128, 1024], bf16, tag="tp2")
                    for k in range(8):
                        t = 8 * h + k
                        nc.tensor.transpose(ptx[:, 128 * k:128 * (k + 1)],
                                            xhat[:, t, :], ident[:])
                    nc.scalar.copy(out=xhatT[:, 1024 * h:1024 * (h + 1)],
                                   in_=ptx[:])
                for h in range(2):
                    ptd = tpsum.tile([128, 1024], bf16, tag="tp2")
                    for k in range(8):
                        t = 8 * h + k
                        nc.tensor.transpose(ptd[:, 128 * k:128 * (k + 1)],
                                            ydn[:, t, :], ident[:])
                    nc.vector.tensor_copy(
                        out=ydT[:, 1024 * h:1024 * (h + 1)], in_=ptd[:])
                for g in range(YCH):
                    pty = tpsum.tile([128, 2048], bf16, tag="tp4")
                    for k in range(YCT):
                        t = YCT * g + k
                        nc.tensor.transpose(pty[:, 128 * k:128 * (k + 1)],
                                            ynat[:, t, :], ident[:])
                    nc.vector.tensor_copy(
                        out=yT[:, YCT * g:YCT * (g + 1), :]
                        .rearrange("p a b -> p (a b)"),
                        in_=pty[:])

            # ---- chunk-0 y norms tail (DVE reduce + rsqrt)
            with nc.allow_low_precision("norm sums in bf16 are plenty"):
                ny2h0 = scr.tile([128, YCT], bf16, tag="nyh", name="nyh0")
                nc.vector.tensor_reduce(out=ny2h0[:], in_=ysq[0][:],
                                        axis=AX.X, op=ALU.add)
                nc.vector.tensor_copy(out=ny2[:, 0:YCT], in_=ny2h0[:])
            nc.vector.reciprocal(t2y[:, 0:YCT], ny2[:, 0:YCT])
            nc.scalar.sqrt(rny[:, 0:YCT], t2y[:, 0:YCT])

            # ---- main loop
            with (
                tc.tile_pool(name="mpa", bufs=2, space="PSUM") as mpa,
                tc.tile_pool(name="mpd", bufs=2, space="PSUM") as mpd,
            ):
                for t in range(TYJ):
                    if t == MID_AT:
                        # diag products, emitted mid-stream (SBUF only)
                        nc.vector.tensor_mul(prodT[:], xhatT[:], ydT[:])
                        with nc.allow_low_precision("diag bf16"):
                            ydsqT = scr.tile([128, TXI * 128], bf16,
                                             tag="sqd", name="ydsqT")
                            nc.gpsimd.tensor_mul(ydsqT[:], ydT[:], ydT[:])
                    if t == RNY1_AT:
                        # chunk-1 y norms tail
                        with nc.allow_low_precision("norm bf16"):
                            ny2h1 = scr.tile([128, YCT], bf16, tag="nyh",
                                             name="nyh1")
                            nc.vector.tensor_reduce(out=ny2h1[:],
                                                    in_=ysq[1][:],
                                                    axis=AX.X, op=ALU.add)
                            nc.vector.tensor_copy(out=ny2[:, YCT:TYJ],
                                                  in_=ny2h1[:])
                        nc.vector.reciprocal(t2y[:, YCT:TYJ],
                                             ny2[:, YCT:TYJ])
                        nc.scalar.sqrt(rny[:, YCT:TYJ], t2y[:, YCT:TYJ])

                    lhsT = yT[:, t, :]
                    pa = mpa.tile([128, ACW], f32, tag="pa")
                    pd = mpd.tile([128, 2048 - ACW], f32, tag="pd")
                    for k in range(4):
                        col = 512 * k
                        dst = (pa[:, col:col + 512] if col < ACW
                               else pd[:, col - ACW:col - ACW + 512])
                        nc.tensor.matmul(dst, lhsT,
                                         xhatT[:, col:col + 512])
                    nc.scalar.activation(
                        dumpA[:], pa[:], AF.Relu,
                        scale=rny[:, t:t + 1],
                        accum_out=R[:, 2 * t:2 * t + 1])
                    nc.vector.tensor_scalar(
                        out=dumpD[:], in0=pd[:],
                        scalar1=0.0, scalar2=None,
                        op0=ALU.max, op1=ALU.add,
                        accum_out=R[:, 2 * t + 1:2 * t + 2])

            # post-scale DVE R columns (odd) by rny
            nc.vector.tensor_mul(R[:, 1:64:2], R[:, 1:64:2], rny[:, 0:TYJ])

            # ---- diag sums via ones-matmuls (after main pools close)
            with tc.tile_pool(name="npsum", bufs=1, space="PSUM") as npsum:
                pn = npsum.tile([128, 64], f32, tag="pn")
                for t in range(TXI):       # d2 -> cols 0:16
                    nc.tensor.matmul(pn[:, t:t + 1],
                                     prodT[:, 128 * t:128 * (t + 1)],
                                     onecol[:])
                for t in range(TXI):       # nyd2 -> cols 16:32
                    nc.tensor.matmul(pn[:, 16 + t:17 + t],
                                     ydsqT[:, 128 * t:128 * (t + 1)],
                                     onecol[:])
                nc.vector.tensor_copy(out=nd64[:, 0:32], in_=pn[:, 0:32])

            nc.vector.reciprocal(t1x[:], nd64[:, 16:32])
            nc.scalar.sqrt(rnyd[:], t1x[:])
            nc.vector.tensor_mul(sim_d[:], nd64[:, 0:16], rnyd[:])
            nc.scalar.activation(relu_d[:], sim_d[:], AF.Relu)
            nc.vector.scalar_tensor_tensor(
                out=scr.tile([128, TXI], f32, tag="dd", name="dd")[:],
                in0=sim_d[:], scalar=1.0, in1=relu_d[:],
                op0=ALU.mult, op1=ALU.add, accum_out=outsb[:, 1:2])

            # ---- final: sum R columns
            nc.vector.tensor_reduce(out=outsb[:, 0:1], in_=R[:],
                                    axis=AX.X, op=ALU.add)
            nc.sync.dma_start(out=out_d[:], in_=outsb[:])

    nc.compile()
    _CACHE["nc"] = nc
    return nc


# cores whose x block lies inside their y range own the diag correction
_DIAG_OWNER = [1, 0, 1, 0, 0, 1, 0, 1]


def _in_maps(x, y):
    maps = []
    for c in range(NCORES):
        bi, bj = c // 2, c % 2
        xsl = slice(XI * bi, XI * (bi + 1))
        ysl = slice(YJ * bj, YJ * (bj + 1))
        maps.append({"xs": np.ascontiguousarray(x[xsl]),
                     "y": np.ascontiguousarray(y[ysl]),
                     "yd": np.ascontiguousarray(y[xsl])})
    return maps


def _combine(results):
    total = 0.0
    for c in range(NCORES):
        o = results[c]["out"].astype(np.float64)
        total += o[:, 0].sum()
        if _DIAG_OWNER[c]:
            total += XI - o[:, 1].sum()
    return np.float32(total / (float(N) * float(N)))


def _run(x, y, trace=False):
    nc = _build()
    res = run_bass_kernel_spmd(nc, _in_maps(x, y), list(range(NCORES)),
                               trace=trace)
    return _combine(res.results), res


def kernel(x, y):
    x = np.asarray(x, dtype=np.float32)
    y = np.asarray(y, dtype=np.float32)
    loss, _ = _run(x, y, trace=False)
    return loss


# revision 14
# speedup vs baseline: 1.0049x; 1.0049x over previous
"""CrossCosineEmbeddingLoss kernel for 8 trn2 NeuronCores (v8).

loss = mean over all (i,j) of: 1 - cos(x_i, y_j) if i==j else relu(cos(x_i, y_j))

Identity:  total = sum_ij relu(sim_ij) + sum_i (1 - sim_ii - relu(sim_ii))

Sharding (2x4 grid): core c = (bi, bj), bi = c // 2, bj = c % 2.
  x rows [2048*bi, 2048*(bi+1)) x y rows [4096*bj, 4096*(bj+1)).
Each core computes sum_ij relu(x_hat_i . y_j) / ||y_j|| over its block.
Diag correction only used from cores whose x block lies in their y range.

Scheduling notes (engine queues are in-order, Tile reorders a little):
 - GpSimd tensor ops hold the shared SBUF port and block DVE 2-port ops,
   so DVE sticks to single-port ops (2x_1P tensor_tensor, tensor_reduce,
   PSUM-source ops) while GpSimd squares run.
 - DVE is the convoy engine: ydT copies go to ACT; ny2 reduce writes f32
   directly; diag column sums are N=1 ones-matmuls on the PE reusing the
   main PSUM pool tags after the last tile.

Per-core pipeline:
  - x: SWDGE cast-DMA bf16 (2 halves); DVE TT square + segmented reduce;
    rsqrt; DVE tensor_scalar row scale; PE transpose; ACT copy -> x_hatT
  - y: SWDGE cast-DMA bf16 (2 chunks); GpSimd natural squares + DVE
    reduce + rsqrt -> rny (chunk 1 tail emitted mid-main-loop);
    PE transpose + DVE copy -> yT
  - yd: cast-DMA last; PE transpose; ACT copies -> ydT; prodT/ydsqT
    squares mid-loop (DVE 2x_1P / GpSimd)
  - main: 32 j-tiles of [128j, 2048i] fp32 PSUM; per tile two 2-bank
    pool tiles (bufs=2 each): ACT activation(Relu, scale=rny, accum_out)
    / DVE tensor_scalar(max 0, accum_out) -> R, outputs to SBUF dumps
  - diag tail: 32 ones-matmuls into reused pool tiles, tiny fp32 ops
Host combines [128,2] partials; diag col used only from owner cores.
"""

import numpy as np

import concourse.bacc as bacc
import concourse.bass as bass
import concourse.tile as tile
from concourse import mybir
from concourse.bass_utils import run_bass_kernel_spmd
from concourse.masks import make_identity

N, D = 8192, 128
NCORES = 8
XI = 2048            # x rows per core
YJ = 4096            # y rows per core
TXI = XI // 128      # 16 x tiles
TYJ = YJ // 128      # 32 y j-tiles
YCH = 2              # y cast-DMA chunks
YCT = TYJ // YCH     # 16 j-tiles per chunk
ACW = 1024           # ACT's share of each [128, 2048] tile (bank aligned)
RNY1_AT = 8          # emit chunk-1 norm tail before this main tile
MID_AT = 6           # emit diag squares before this main tile

f32 = mybir.dt.float32
bf16 = mybir.dt.bfloat16
AF = mybir.ActivationFunctionType
ALU = mybir.AluOpType
AX = mybir.AxisListType

_CACHE = {}


def _build():
    if "nc" in _CACHE:
        return _CACHE["nc"]
    nc = bacc.Bacc("TRN2", target_bir_lowering=False, debug=False,
                   num_devices=NCORES)
    xs_d = nc.dram_tensor("xs", [XI, D], f32, kind="ExternalInput")
    y_d = nc.dram_tensor("y", [YJ, D], f32, kind="ExternalInput")
    yd_d = nc.dram_tensor("yd", [XI, D], f32, kind="ExternalInput")
    out_d = nc.dram_tensor("out", [128, 2], f32, kind="ExternalOutput")

    with tile.TileContext(nc) as tc:
        with (
            tc.tile_pool(name="singles", bufs=1) as singles,
            tc.tile_pool(name="scr", bufs=2) as scr,
        ):
            ident = singles.tile([128, 128], bf16)
            make_identity(nc, ident[:])
            onecol = singles.tile([128, 1], bf16)
            nc.vector.memset(onecol[:], 1.0)
            warm = singles.tile([128, 1], f32)
            nc.vector.memset(warm[:], 1.0)
            nc.scalar.sqrt(warm[:], warm[:])   # preload sqrt table set early

            ynat = singles.tile([128, TYJ, 128], bf16)   # row j: 2048g+16p+t
            yT = singles.tile([128, TYJ, 128], bf16)     # [d, t, j-col]
            xnat = singles.tile([128, TXI, 128], bf16)   # row i: 1024h+8p+t
            xhat = singles.tile([128, TXI, 128], bf16)
            xhatT = singles.tile([128, TXI * 128], bf16)
            ydn = singles.tile([128, TXI, 128], bf16)    # same layout as x
            ydT = singles.tile([128, TXI * 128], bf16)
            prodT = singles.tile([128, TXI * 128], bf16)
            ydsqT = singles.tile([128, TXI * 128], bf16)

            ny2 = singles.tile([128, TYJ], f32)
            t2y = singles.tile([128, TYJ], f32)
            rny = singles.tile([128, TYJ], f32)
            nx2 = singles.tile([128, TXI], f32)
            t1x = singles.tile([128, TXI], f32)
            rnx = singles.tile([128, TXI], f32)
            nd32 = singles.tile([128, 32], f32)   # d2 | nyd2
            rnyd = singles.tile([128, TXI], f32)
            sim_d = singles.tile([128, TXI], f32)
            relu_d = singles.tile([128, TXI], f32)
            R = singles.tile([128, 64], f32)
            outsb = singles.tile([128, 2], f32)
            dumpA = singles.tile([128, ACW], bf16)
            dumpD = singles.tile([128, 2048 - ACW], bf16)

            # ---- input DMAs (SWDGE cast fp32->bf16): x halves, y0, y1, yd
            for h in range(2):
                rows = slice(1024 * h, 1024 * (h + 1))
                nc.gpsimd.dma_start(
                    out=xnat[:, 8 * h:8 * (h + 1), :],
                    in_=xs_d[rows].rearrange("(p t) d -> p t d", t=8))
            for g in range(YCH):
                rows = slice(2048 * g, 2048 * (g + 1))
                nc.gpsimd.dma_start(
                    out=ynat[:, YCT * g:YCT * (g + 1), :],
                    in_=y_d[rows].rearrange("(p t) d -> p t d", t=YCT))
            for h in range(2):
                rows = slice(1024 * h, 1024 * (h + 1))
                nc.gpsimd.dma_start(
                    out=ydn[:, 8 * h:8 * (h + 1), :],
                    in_=yd_d[rows].rearrange("(p t) d -> p t d", t=8))

            # ---- x norms + scale (per half); single-port DVE ops
            with nc.allow_low_precision("norm sums in bf16 are plenty"):
                for h in range(2):
                    hs = slice(8 * h, 8 * (h + 1))
                    xsq = scr.tile([128, 8, 128], bf16, tag="xsq",
                                   name=f"xsq{h}")
                    nc.vector.tensor_mul(
                        xsq[:].rearrange("p a b -> p (a b)"),
                        xnat[:, hs, :].rearrange("p a b -> p (a b)"),
                        xnat[:, hs, :].rearrange("p a b -> p (a b)"))
                    nc.vector.tensor_reduce(out=nx2[:, hs], in_=xsq[:],
                                            axis=AX.X, op=ALU.add)
                    nc.vector.reciprocal(t1x[:, hs], nx2[:, hs])
                    nc.scalar.sqrt(rnx[:, hs], t1x[:, hs])   # 1/||x_i||
                    for t in range(8 * h, 8 * h + 8):
                        nc.vector.tensor_scalar(
                            out=xhat[:, t, :], in0=xnat[:, t, :],
                            scalar1=rnx[:, t:t + 1], scalar2=None,
                            op0=ALU.mult)

            # ---- GpSimd natural squares for y norms (both chunks)
            ysq = []
            with nc.allow_low_precision("norm sums in bf16 are plenty"):
                for g in range(YCH):
                    s = scr.tile([128, YCT, 128], bf16, tag=f"sq{g}",
                                 name=f"ysq{g}")
                    gs = slice(YCT * g, YCT * (g + 1))
                    nc.gpsimd.tensor_mul(
                        s[:].rearrange("p a b -> p (a b)"),
                        ynat[:, gs, :].rearrange("p a b -> p (a b)"),
                        ynat[:, gs, :].rearrange("p a b -> p (a b)"))
                    ysq.append(s)

            # ---- transposes on PE (bf16) + copies to SBUF
            with tc.tile_pool(name="tpsum", bufs=2, space="PSUM") as tpsum:
                for h in range(2):
                    ptx = tpsum.tile([128, 1024], bf16, tag="tp2")
                    for k in range(8):
                        t = 8 * h + k
                        nc.tensor.transpose(ptx[:, 128 * k:128 * (k + 1)],
                                            xhat[:, t, :], ident[:])
                    nc.scalar.copy(out=xhatT[:, 1024 * h:1024 * (h + 1)],
                                   in_=ptx[:])
                for g in range(YCH):
                    pty = tpsum.tile([128, 2048], bf16, tag="tp4")
                    for k in range(YCT):
                        t = YCT * g + k
                        nc.tensor.transpose(pty[:, 128 * k:128 * (k + 1)],
                                            ynat[:, t, :], ident[:])
                    nc.vector.tensor_copy(
                        out=yT[:, YCT * g:YCT * (g + 1), :]
                        .rearrange("p a b -> p (a b)"),
                        in_=pty[:])
                for h in range(2):
                    ptd = tpsum.tile([128, 1024], bf16, tag="tp2")
                    for k in range(8):
                        t = 8 * h + k
                        nc.tensor.transpose(ptd[:, 128 * k:128 * (k + 1)],
                                            ydn[:, t, :], ident[:])
                    nc.scalar.copy(out=ydT[:, 1024 * h:1024 * (h + 1)],
                                   in_=ptd[:])

            # ---- chunk-0 y norms tail (DVE reduce -> f32, rsqrt)
            nc.vector.tensor_reduce(out=ny2[:, 0:YCT], in_=ysq[0][:],
                                    axis=AX.X, op=ALU.add)
            nc.vector.reciprocal(t2y[:, 0:YCT], ny2[:, 0:YCT])
            nc.scalar.sqrt(rny[:, 0:YCT], t2y[:, 0:YCT])

            # ---- main loop
            with (
                tc.tile_pool(name="mpa", bufs=2, space="PSUM") as mpa,
                tc.tile_pool(name="mpd", bufs=2, space="PSUM") as mpd,
            ):
                for t in range(TYJ):
                    if t == MID_AT:
                        # diag squares, emitted mid-stream (SBUF only)
                        nc.vector.tensor_mul(prodT[:], xhatT[:], ydT[:])
                        nc.gpsimd.tensor_mul(ydsqT[:], ydT[:], ydT[:])
                    if t == RNY1_AT:
                        # chunk-1 y norms tail
                        nc.vector.tensor_reduce(out=ny2[:, YCT:TYJ],
                                                in_=ysq[1][:],
                                                axis=AX.X, op=ALU.add)
                        nc.vector.reciprocal(t2y[:, YCT:TYJ],
                                             ny2[:, YCT:TYJ])
                        nc.scalar.sqrt(rny[:, YCT:TYJ], t2y[:, YCT:TYJ])

                    lhsT = yT[:, t, :]
                    pa = mpa.tile([128, ACW], f32, tag="pa")
                    pd = mpd.tile([128, 2048 - ACW], f32, tag="pd")
                    for k in range(4):
                        col = 512 * k
                        dst = (pa[:, col:col + 512] if col < ACW
                               else pd[:, col - ACW:col - ACW + 512])
                        nc.tensor.matmul(dst, lhsT,
                                         xhatT[:, col:col + 512])
                    nc.scalar.activation(
                        dumpA[:], pa[:], AF.Relu,
                        scale=rny[:, t:t + 1],
                        accum_out=R[:, 2 * t:2 * t + 1])
                    nc.vector.tensor_scalar(
                        out=dumpD[:], in0=pd[:],
                        scalar1=0.0, scalar2=None,
                        op0=ALU.max, op1=ALU.add,
                        accum_out=R[:, 2 * t + 1:2 * t + 2])

                # ---- diag column sums: ones-matmuls into reused pool tiles
                pna = mpa.tile([128, ACW], f32, tag="pa")
                for t in range(TXI):       # d2
                    nc.tensor.matmul(pna[:, t:t + 1],
                                     prodT[:, 128 * t:128 * (t + 1)],
                                     onecol[:])
                pnd = mpd.tile([128, 2048 - ACW], f32, tag="pd")
                for t in range(TXI):       # nyd2
                    nc.tensor.matmul(pnd[:, t:t + 1],
                                     ydsqT[:, 128 * t:128 * (t + 1)],
                                     onecol[:])
                nc.vector.tensor_copy(out=nd32[:, 0:16], in_=pna[:, 0:16])
                nc.vector.tensor_copy(out=nd32[:, 16:32], in_=pnd[:, 0:16])

            # post-scale DVE R columns (odd) by rny
            nc.vector.tensor_mul(R[:, 1:64:2], R[:, 1:64:2], rny[:, 0:TYJ])

            # ---- diag scalars
            nc.vector.reciprocal(t1x[:], nd32[:, 16:32])
            nc.scalar.sqrt(rnyd[:], t1x[:])
            nc.vector.tensor_mul(sim_d[:], nd32[:, 0:16], rnyd[:])
            nc.scalar.activation(relu_d[:], sim_d[:], AF.Relu)
            nc.vector.scalar_tensor_tensor(
                out=scr.tile([128, TXI], f32, tag="dd", name="dd")[:],
                in0=sim_d[:], scalar=1.0, in1=relu_d[:],
                op0=ALU.mult, op1=ALU.add, accum_out=outsb[:, 1:2])

            # ---- final: sum R columns
            nc.vector.tensor_reduce(out=outsb[:, 0:1], in_=R[:],
                                    axis=AX.X, op=ALU.add)
            nc.sync.dma_start(out=out_d[:], in_=outsb[:])

    nc.compile()
    _CACHE["nc"] = nc
    return nc


# cores whose x block lies inside their y range own the diag correction
_DIAG_OWNER = [1, 0, 1, 0, 0, 1, 0, 1]


def _in_maps(x, y):
    maps = []
    for c in range(NCORES):
        bi, bj = c // 2, c % 2
        xsl = slice(XI * bi, XI * (bi + 1))
        ysl = slice(YJ * bj, YJ * (bj + 1))
        maps.append({"xs": np.ascontiguousarray(x[xsl]),
                     "y": np.ascontiguousarray(y[ysl]),
                     "yd": np.ascontiguousarray(y[xsl])})
    return maps


def _combine(results):
    total = 0.0
    for c in range(NCORES):
        o = results[c]["out"].astype(np.float64)
        total += o[:, 0].sum()
        if _DIAG_OWNER[c]:
            total += XI - o[:, 1].sum()
    return np.float32(total / (float(N) * float(N)))


def _run(x, y, trace=False):
    nc = _build()
    res = run_bass_kernel_spmd(nc, _in_maps(x, y), list(range(NCORES)),
                               trace=trace)
    return _combine(res.results), res


def kernel(x, y):
    x = np.asarray(x, dtype=np.float32)
    y = np.asarray(y, dtype=np.float32)
    loss, _ = _run(x, y, trace=False)
    return loss


# revision 16
# speedup vs baseline: 1.0142x; 1.0092x over previous
"""CrossCosineEmbeddingLoss kernel for 8 trn2 NeuronCores (v8).

loss = mean over all (i,j) of: 1 - cos(x_i, y_j) if i==j else relu(cos(x_i, y_j))

Identity:  total = sum_ij relu(sim_ij) + sum_i (1 - sim_ii - relu(sim_ii))

Sharding (2x4 grid): core c = (bi, bj), bi = c // 2, bj = c % 2.
  x rows [2048*bi, 2048*(bi+1)) x y rows [4096*bj, 4096*(bj+1)).
Each core computes sum_ij relu(x_hat_i . y_j) / ||y_j|| over its block.
Diag correction only used from cores whose x block lies in their y range.

Scheduling notes (engine queues are in-order, Tile reorders a little):
 - GpSimd tensor ops hold the shared SBUF port and block DVE 2-port ops,
   so DVE sticks to single-port ops (2x_1P tensor_tensor, tensor_reduce,
   PSUM-source ops) while GpSimd squares run.
 - DVE is the convoy engine: ydT copies go to ACT; ny2 reduce writes f32
   directly; diag column sums are N=1 ones-matmuls on the PE reusing the
   main PSUM pool tags after the last tile.

Per-core pipeline:
  - x: SWDGE cast-DMA bf16 (2 halves); DVE TT square + segmented reduce;
    rsqrt; DVE tensor_scalar row scale; PE transpose; ACT copy -> x_hatT
  - y: SWDGE cast-DMA bf16 (2 chunks); GpSimd natural squares + DVE
    reduce + rsqrt -> rny (chunk 1 tail emitted mid-main-loop);
    PE transpose + DVE copy -> yT
  - yd: cast-DMA last; PE transpose; ACT copies -> ydT; prodT/ydsqT
    squares mid-loop (DVE 2x_1P / GpSimd)
  - main: 32 j-tiles of [128j, 2048i] fp32 PSUM; per tile two 2-bank
    pool tiles (bufs=2 each): ACT activation(Relu, scale=rny, accum_out)
    / DVE tensor_scalar(max 0, accum_out) -> R, outputs to SBUF dumps
  - diag tail: 32 ones-matmuls into reused pool tiles, tiny fp32 ops
Host combines [128,2] partials; diag col used only from owner cores.
"""

import numpy as np

import concourse.bacc as bacc
import concourse.bass as bass
import concourse.tile as tile
from concourse import mybir
from concourse.bass_utils import run_bass_kernel_spmd
from concourse.masks import make_identity

N, D = 8192, 128
NCORES = 8
XI = 2048            # x rows per core
YJ = 4096            # y rows per core
TXI = XI // 128      # 16 x tiles
TYJ = YJ // 128      # 32 y j-tiles
YCH = 2              # y cast-DMA chunks
YCT = TYJ // YCH     # 16 j-tiles per chunk
ACW = 1024           # ACT's share of each [128, 2048] tile (bank aligned)
RNY1_AT = 8          # emit chunk-1 norm tail before this main tile
MID_AT = 6           # emit diag squares before this main tile

f32 = mybir.dt.float32
bf16 = mybir.dt.bfloat16
AF = mybir.ActivationFunctionType
ALU = mybir.AluOpType
AX = mybir.AxisListType

_CACHE = {}


def _build():
    if "nc" in _CACHE:
        return _CACHE["nc"]
    nc = bacc.Bacc("TRN2", target_bir_lowering=False, debug=False,
                   num_devices=NCORES)
    xs_d = nc.dram_tensor("xs", [XI, D], f32, kind="ExternalInput")
    y_d = nc.dram_tensor("y", [YJ, D], f32, kind="ExternalInput")
    yd_d = nc.dram_tensor("yd", [XI, D], f32, kind="ExternalInput")
    out_d = nc.dram_tensor("out", [128, 2], f32, kind="ExternalOutput")

    with tile.TileContext(nc) as tc:
        with (
            tc.tile_pool(name="singles", bufs=1) as singles,
            tc.tile_pool(name="scr", bufs=2) as scr,
        ):
            ident = singles.tile([128, 128], bf16)
            make_identity(nc, ident[:])
            warm = singles.tile([128, 1], f32)
            nc.vector.memset(warm[:], 1.0)
            nc.scalar.sqrt(warm[:], warm[:])   # preload sqrt table set early

            ynat = singles.tile([128, TYJ, 128], bf16)   # row j: 2048g+16p+t
            yT = singles.tile([128, TYJ, 128], bf16)     # [d, t, j-col]
            xnat = singles.tile([128, TXI, 128], bf16)   # row i: 1024h+8p+t
            xhat = singles.tile([128, TXI, 128], bf16)
            xhatT = singles.tile([128, TXI * 128], bf16)
            ydn = singles.tile([128, TXI, 128], bf16)    # same layout as x

            ny2 = singles.tile([128, TYJ], f32)
            t2y = singles.tile([128, TYJ], f32)
            rny = singles.tile([128, TYJ], f32)
            nx2 = singles.tile([128, TXI], f32)
            t1x = singles.tile([128, TXI], f32)
            rnx = singles.tile([128, TXI], f32)
            d2 = singles.tile([128, TXI], f32)
            nyd2 = singles.tile([128, TXI], f32)
            rnyd = singles.tile([128, TXI], f32)
            sim_d = singles.tile([128, TXI], f32)
            relu_d = singles.tile([128, TXI], f32)
            R = singles.tile([128, 64], f32)
            outsb = singles.tile([128, 2], f32)
            dumpA = singles.tile([128, ACW], bf16)
            dumpD = singles.tile([128, 2048 - ACW], bf16)

            # ---- input DMAs (SWDGE cast fp32->bf16): x halves, y0, y1, yd
            for h in range(2):
                rows = slice(1024 * h, 1024 * (h + 1))
                nc.gpsimd.dma_start(
                    out=xnat[:, 8 * h:8 * (h + 1), :],
                    in_=xs_d[rows].rearrange("(p t) d -> p t d", t=8))
            for g in range(YCH):
                rows = slice(2048 * g, 2048 * (g + 1))
                nc.gpsimd.dma_start(
                    out=ynat[:, YCT * g:YCT * (g + 1), :],
                    in_=y_d[rows].rearrange("(p t) d -> p t d", t=YCT))
            for h in range(2):
                rows = slice(1024 * h, 1024 * (h + 1))
                nc.gpsimd.dma_start(
                    out=ydn[:, 8 * h:8 * (h + 1), :],
                    in_=yd_d[rows].rearrange("(p t) d -> p t d", t=8))

            # ---- x norms + scale (per half); single-port DVE ops
            with nc.allow_low_precision("norm sums in bf16 are plenty"):
                for h in range(2):
                    hs = slice(8 * h, 8 * (h + 1))
                    xsq = scr.tile([128, 8, 128], bf16, tag="xsq",
                                   name=f"xsq{h}")
                    nc.vector.tensor_mul(
                        xsq[:].rearrange("p a b -> p (a b)"),
                        xnat[:, hs, :].rearrange("p a b -> p (a b)"),
                        xnat[:, hs, :].rearrange("p a b -> p (a b)"))
                    nc.vector.tensor_reduce(out=nx2[:, hs], in_=xsq[:],
                                            axis=AX.X, op=ALU.add)
                    nc.vector.reciprocal(t1x[:, hs], nx2[:, hs])
                    nc.scalar.sqrt(rnx[:, hs], t1x[:, hs])   # 1/||x_i||
                    for t in range(8 * h, 8 * h + 8):
                        nc.vector.tensor_scalar(
                            out=xhat[:, t, :], in0=xnat[:, t, :],
                            scalar1=rnx[:, t:t + 1], scalar2=None,
                            op0=ALU.mult)

            # ---- GpSimd natural squares for y norms (both chunks)
            ysq = []
            with nc.allow_low_precision("norm sums in bf16 are plenty"):
                for g in range(YCH):
                    s = scr.tile([128, YCT, 128], bf16, tag=f"sq{g}",
                                 name=f"ysq{g}")
                    gs = slice(YCT * g, YCT * (g + 1))
                    nc.gpsimd.tensor_mul(
                        s[:].rearrange("p a b -> p (a b)"),
                        ynat[:, gs, :].rearrange("p a b -> p (a b)"),
                        ynat[:, gs, :].rearrange("p a b -> p (a b)"))
                    ysq.append(s)

            # ---- transposes on PE (bf16) + copies to SBUF
            with tc.tile_pool(name="tpsum", bufs=2, space="PSUM") as tpsum:
                for h in range(2):
                    ptx = tpsum.tile([128, 1024], bf16, tag="tp2")
                    for k in range(8):
                        t = 8 * h + k
                        nc.tensor.transpose(ptx[:, 128 * k:128 * (k + 1)],
                                            xhat[:, t, :], ident[:])
                    nc.scalar.copy(out=xhatT[:, 1024 * h:1024 * (h + 1)],
                                   in_=ptx[:])
                for g in range(YCH):
                    pty = tpsum.tile([128, 2048], bf16, tag="tp4")
                    for k in range(YCT):
                        t = YCT * g + k
                        nc.tensor.transpose(pty[:, 128 * k:128 * (k + 1)],
                                            ynat[:, t, :], ident[:])
                    nc.vector.tensor_copy(
                        out=yT[:, YCT * g:YCT * (g + 1), :]
                        .rearrange("p a b -> p (a b)"),
                        in_=pty[:])

            # ---- chunk-0 y norms tail (DVE reduce -> f32, rsqrt)
            nc.vector.tensor_reduce(out=ny2[:, 0:YCT], in_=ysq[0][:],
                                    axis=AX.X, op=ALU.add)
            nc.vector.reciprocal(t2y[:, 0:YCT], ny2[:, 0:YCT])
            nc.scalar.sqrt(rny[:, 0:YCT], t2y[:, 0:YCT])

            # ---- main loop
            with (
                tc.tile_pool(name="mpa", bufs=2, space="PSUM") as mpa,
                tc.tile_pool(name="mpd", bufs=2, space="PSUM") as mpd,
            ):
                for t in range(TYJ):
                    if t == MID_AT:
                        # diag squares on GpSimd (idle mid-loop, SBUF only)
                        prodn = scr.tile([128, TXI, 128], bf16, tag="gp",
                                         name="prodn")
                        nc.gpsimd.tensor_mul(
                            prodn[:].rearrange("p a b -> p (a b)"),
                            xhat[:].rearrange("p a b -> p (a b)"),
                            ydn[:].rearrange("p a b -> p (a b)"))
                        ydsqn = scr.tile([128, TXI, 128], bf16, tag="gq",
                                         name="ydsqn")
                        nc.gpsimd.tensor_mul(
                            ydsqn[:].rearrange("p a b -> p (a b)"),
                            ydn[:].rearrange("p a b -> p (a b)"),
                            ydn[:].rearrange("p a b -> p (a b)"))
                    if t == 24:
                        nc.vector.tensor_reduce(out=d2[:], in_=prodn[:],
                                                axis=AX.X, op=ALU.add)
                    if t == 28:
                        nc.vector.tensor_reduce(out=nyd2[:], in_=ydsqn[:],
                                                axis=AX.X, op=ALU.add)
                    if t == RNY1_AT:
                        # chunk-1 y norms tail
                        nc.vector.tensor_reduce(out=ny2[:, YCT:TYJ],
                                                in_=ysq[1][:],
                                                axis=AX.X, op=ALU.add)
                        nc.vector.reciprocal(t2y[:, YCT:TYJ],
                                             ny2[:, YCT:TYJ])
                        nc.scalar.sqrt(rny[:, YCT:TYJ], t2y[:, YCT:TYJ])

                    lhsT = yT[:, t, :]
                    pa = mpa.tile([128, ACW], f32, tag="pa")
                    pd = mpd.tile([128, 2048 - ACW], f32, tag="pd")
                    for k in range(4):
                        col = 512 * k
                        dst = (pa[:, col:col + 512] if col < ACW
                               else pd[:, col - ACW:col - ACW + 512])
                        nc.tensor.matmul(dst, lhsT,
                                         xhatT[:, col:col + 512])
                    nc.scalar.activation(
                        dumpA[:], pa[:], AF.Relu,
                        scale=rny[:, t:t + 1],
                        accum_out=R[:, 2 * t:2 * t + 1])
                    nc.vector.tensor_scalar(
                        out=dumpD[:], in0=pd[:],
                        scalar1=0.0, scalar2=None,
                        op0=ALU.max, op1=ALU.add,
                        accum_out=R[:, 2 * t + 1:2 * t + 2])

            # post-scale DVE R columns (odd) by rny
            nc.vector.tensor_mul(R[:, 1:64:2], R[:, 1:64:2], rny[:, 0:TYJ])

            # ---- diag scalars
            nc.vector.reciprocal(t1x[:], nyd2[:])
            nc.scalar.sqrt(rnyd[:], t1x[:])
            nc.vector.tensor_mul(sim_d[:], d2[:], rnyd[:])
            nc.scalar.activation(relu_d[:], sim_d[:], AF.Relu)
            nc.vector.scalar_tensor_tensor(
                out=scr.tile([128, TXI], f32, tag="dd", name="dd")[:],
                in0=sim_d[:], scalar=1.0, in1=relu_d[:],
                op0=ALU.mult, op1=ALU.add, accum_out=outsb[:, 1:2])

            # ---- final: sum R columns
            nc.vector.tensor_reduce(out=outsb[:, 0:1], in_=R[:],
                                    axis=AX.X, op=ALU.add)
            nc.sync.dma_start(out=out_d[:], in_=outsb[:])

    nc.compile()
    _CACHE["nc"] = nc
    return nc


# cores whose x block lies inside their y range own the diag correction
_DIAG_OWNER = [1, 0, 1, 0, 0, 1, 0, 1]


def _in_maps(x, y):
    maps = []
    for c in range(NCORES):
        bi, bj = c // 2, c % 2
        xsl = slice(XI * bi, XI * (bi + 1))
        ysl = slice(YJ * bj, YJ * (bj + 1))
        maps.append({"xs": np.ascontiguousarray(x[xsl]),
                     "y": np.ascontiguousarray(y[ysl]),
                     "yd": np.ascontiguousarray(y[xsl])})
    return maps


def _combine(results):
    total = 0.0
    for c in range(NCORES):
        o = results[c]["out"].astype(np.float64)
        total += o[:, 0].sum()
        if _DIAG_OWNER[c]:
            total += XI - o[:, 1].sum()
    return np.float32(total / (float(N) * float(N)))


def _run(x, y, trace=False):
    nc = _build()
    res = run_bass_kernel_spmd(nc, _in_maps(x, y), list(range(NCORES)),
                               trace=trace)
    return _combine(res.results), res


def kernel(x, y):
    x = np.asarray(x, dtype=np.float32)
    y = np.asarray(y, dtype=np.float32)
    loss, _ = _run(x, y, trace=False)
    return loss
